# revision 5
# baseline (speedup 1.0000x reference)
"""AuxSpatialGather (per-class masked mean pooling) Trainium2 kernel.

Computes, per sample b:  ctx[k, c] = mean over pixels n with gt[n]==k of feats[c, n]
(classes with zero pixels get 0), returned as [B, C, K, 1] float32.

Strategy (8 NeuronCores, data-parallel over batch, 2 samples/core):
  - The kernel is HBM-bound: 64 MiB of feats per core at ~330 GB/s is a
    ~204 us stream that the profile shows runs gapless on the SP HWDGE
    ring. Everything else is organized to hang off that stream with the
    smallest possible un-overlapped head and tail.
  - feats arrive channel-major [C, HW] and are loaded in QUARTER-chunk
    granules [128ch, 1024px] (0.5 MB) so compute readiness tracks the
    stream at ~6 us granularity: the un-overlapped tail after the last
    HBM byte is one quarter's worth of compute (~7 us) instead of a full
    8 MB chunk's (~27 us, with HAM throttling the PE to half rate).
  - fp32 matmul runs at 1/4 rate, so feats are cast f32->f16 (casts
    alternate DVE/ACT so neither engine's queue head-blocks), then
    PE-transposed as PAIRS of f16 pixels viewed as one f32 element
    (transpose-mode is a bit-exact raw mover), evacuated PSUM->SBUF
    (DVE/ACT alternating), and reduced by a one-hot matmul in f16 (two
    parity-split matmuls over a stride-2 rhs view) with fp32 PSUM
    accumulation. Only precision loss: f16 input quantization (~2e-4;
    fp8 was measured at 2.5e-2 end-to-end - over the tolerance).
  - pixel order n = qs*1024 + 8m + 2j + par makes window (qs, j) read
    stride-4 f32 columns of quarter qs only, and lands the gt load in
    contiguous 32-byte runs on the second HWDGE ring (off the feat FIFO).
  - transposes are emitted ci-major within a quarter's 4 windows so PE
    needs only the first channel granule to start a quarter: PE idle
    stays in slivers that don't trip the HAM re-throttle window.
  - per-class counts via a free-dim reduce + ones-vector matmul; the
    final [19, 512] context is scaled by 1/max(cnt,1), transposed to
    [512, 19] on PE, and stored via SWDGE to keep the feat ring clean.
"""

import numpy as np

NUM_CLASSES = 19
B, C, H, W = 16, 512, 128, 128
HW = H * W
N_CORES = 8
S = B // N_CORES  # samples per core
P = 128  # partitions

_compiled = None


def _build_nc(s=S, c=C, hw=HW, qw=1024):
    from concourse import bacc, mybir
    from concourse.tile import TileContext
    from concourse.masks import make_identity

    f32 = mybir.dt.float32
    f16 = mybir.dt.float16
    i32 = mybir.dt.int32
    K = NUM_CLASSES
    n_ci = c // P  # channel granules (4)
    n_g = hw // qw  # quarters per sample (16)
    n_w = 4  # windows (256 pixels) per quarter
    n_t = hw // P  # 128-pixel weight columns per sample (128)
    PF = 3  # load prefetch distance, in quarters
    PFP = 6  # planes (gt) prefetch distance, in quarters

    nc = bacc.Bacc("TRN2", target_bir_lowering=False)
    feats = nc.dram_tensor("feats", [s, c, hw], f32, kind="ExternalInput")
    gt = nc.dram_tensor("gt_seg_map", [s, hw], i32, kind="ExternalInput")
    out = nc.dram_tensor("out", [s, c, K], f32, kind="ExternalOutput")

    with TileContext(nc) as tc:
        with (
            tc.tile_pool(name="const", bufs=1) as const_pool,
            tc.tile_pool(name="stage", bufs=PF + 1) as stage_pool,
            tc.tile_pool(name="quart", bufs=2) as q_pool,
            tc.tile_pool(name="planes", bufs=2) as plane_pool,
            tc.tile_pool(name="ft", bufs=4) as ft_pool,
            tc.tile_pool(name="small", bufs=2) as small_pool,
            tc.tile_pool(name="ftp", bufs=5, space="PSUM") as ftp_pool,
            tc.tile_pool(name="accp", bufs=2, space="PSUM") as acc_pool,
            tc.tile_pool(name="tinyp", bufs=1, space="PSUM") as tiny_pool,
        ):
            ident32 = const_pool.tile([P, P], f32)
            make_identity(nc, ident32[:])
            ones16 = const_pool.tile([P, 1], f16)
            nc.vector.memset(ones16[:], 1.0)

            # Pixel order: n = qs*qw + 8*m + 2*j + par
            # -> G[m, t], t = qs*8 + 2j + par: per-partition runs of 8
            # contiguous gt elements (32B) -> fine DMA on the ACT ring;
            # window (qs, j) reads stride-4 f32 pair-columns of quarter qs.

            def load_quarter(g):
                """4 channel-granule f32 loads for global quarter g."""
                si, qs = g // n_g, g % n_g
                sts = []
                for ci in range(n_ci):
                    st = stage_pool.tile([P, qw], f32, name=f"st{ci}")
                    nc.sync.dma_start(
                        out=st[:],
                        in_=feats[
                            si, ci * P : (ci + 1) * P, qs * qw : (qs + 1) * qw
                        ],
                    )
                    sts.append(st)
                return sts

            def cast_quarter(sts):
                """f32->f16 casts, alternating DVE/ACT."""
                chs = []
                for ci in range(n_ci):
                    ch = q_pool.tile([P, qw], f16, name=f"ch{ci}")
                    if ci % 2 == 0:
                        nc.vector.tensor_copy(ch[:], sts[ci][:])
                    else:
                        nc.scalar.copy(ch[:], sts[ci][:])
                    chs.append(ch)
                return chs

            def build_planes(si):
                """One-hot planes for sample si (quarter-order pixel layout).
                gt DMA on the second HWDGE ring: off the feat FIFO."""
                G_i = plane_pool.tile([P, n_t], i32, name="G_i")
                nc.scalar.dma_start(
                    out=G_i[:].rearrange("p (qs r) -> p qs r", qs=n_g),
                    in_=gt[si].rearrange("(qs p r) -> p qs r", qs=n_g, p=P),
                )
                G_f = plane_pool.tile([P, n_t], f16, name="G_f")
                nc.vector.tensor_copy(G_f[:], G_i[:])
                planes = plane_pool.tile([P, K * n_t], f16, name="planes", tag="pl")
                for k in range(K):
                    nc.vector.tensor_scalar(
                        planes[:, k * n_t : (k + 1) * n_t],
                        G_f[:],
                        float(k),
                        None,
                        op0=mybir.AluOpType.is_equal,
                    )
                return planes

            def build_recip(planes):
                """Per-class counts -> reciprocal [K, 1]."""
                partial = small_pool.tile([P, K], f32, name="partial")
                nc.vector.tensor_reduce(
                    partial[:],
                    planes[:].rearrange("p (k t) -> p k t", k=K),
                    axis=mybir.AxisListType.X,
                    op=mybir.AluOpType.add,
                )
                partial16 = small_pool.tile([P, K], f16, name="partial16")
                nc.vector.tensor_copy(partial16[:], partial[:])
                cnt_ps = tiny_pool.tile([1, K], f32, name="cnt_ps", tag="tiny")
                nc.tensor.matmul(
                    cnt_ps[:], ones16[:], partial16[:], start=True, stop=True
                )
                cnt_sq = small_pool.tile([32, 32], f32, name="cnt_sq")
                nc.vector.memset(cnt_sq[:], 0.0)
                nc.vector.tensor_copy(cnt_sq[:1, :K], cnt_ps[:])
                cnt_tr = small_pool.tile([32, 32], f32, name="cnt_tr")
                nc.vector.transpose(cnt_tr[:], cnt_sq[:])
                recip = small_pool.tile([K, 1], f32, name="recip")
                nc.vector.tensor_scalar_max(recip[:], cnt_tr[:K, :1], 1.0)
                nc.vector.reciprocal(recip[:], recip[:])
                return recip

            # Prime the pipeline: feat loads first (they own the SP ring),
            # then gt + planes for sample 0 on the ACT ring.
            pending = [load_quarter(g) for g in range(PF)]
            planes_cur = build_planes(0)
            n_gq = s * n_g  # total quarters across samples

            for g in range(n_gq):
                si, qs = g // n_g, g % n_g
                if g + PF < n_gq:
                    pending.append(load_quarter(g + PF))
                if qs == n_g - PFP and si + 1 < s:
                    planes_next = build_planes(si + 1)
                if qs == 0:
                    acc = acc_pool.tile([K, c], f32, name="acc")
                    W_all = planes_cur[:].rearrange("p (k t) -> p t k", t=n_t)
                    recip = build_recip(planes_cur)
                chs = cast_quarter(pending.pop(0))

                ftps = [
                    ftp_pool.tile([P, c], f32, name=f"ftp{j}", tag="ftp")
                    for j in range(n_w)
                ]
                for ci in range(n_ci):
                    src = chs[ci][:].bitcast(f32)
                    for j in range(n_w):
                        nc.tensor.transpose(
                            ftps[j][:, ci * P : (ci + 1) * P],
                            src[:, j : j + (P - 1) * n_w + 1 : n_w],
                            ident32[:],
                        )
                for j in range(n_w):
                    fts = ft_pool.tile([P, 2 * c], f16, name="fts")
                    if j % 2 == 0:
                        nc.vector.tensor_copy(fts[:].bitcast(f32), ftps[j][:])
                    else:
                        nc.scalar.copy(fts[:].bitcast(f32), ftps[j][:])
                    fts_pairs = fts[:].rearrange("p (c two) -> p two c", two=2)
                    for par in range(2):
                        t = qs * (2 * n_w) + 2 * j + par
                        nc.tensor.matmul(
                            acc[:],
                            W_all[:, t, :],
                            fts_pairs[:, par, :],
                            start=(t == 0),
                            stop=(t == n_t - 1),
                        )

                if qs == n_g - 1:
                    # ---- normalize + emit [c, K] ----
                    final = small_pool.tile([K, c], f32, name="final")
                    nc.vector.tensor_scalar(
                        final[:], acc[:], recip[:, :1], None,
                        op0=mybir.AluOpType.mult,
                    )
                    outT_ps = tiny_pool.tile(
                        [P, n_ci * K], f32, name="outT_ps", tag="tiny"
                    )
                    for ci in range(n_ci):
                        nc.tensor.transpose(
                            outT_ps[:, ci * K : (ci + 1) * K],
                            final[:K, ci * P : (ci + 1) * P],
                            ident32[:K, :K],
                        )
                    outT = small_pool.tile([P, n_ci * K], f32, name="outT")
                    nc.vector.tensor_copy(outT[:], outT_ps[:])
                    # SWDGE: keep the HWDGE feat-load queue free of DMAs
                    # that wait on compute (FIFO per issuing engine)
                    nc.gpsimd.dma_start(
                        out=out[si].rearrange("(ci p) k -> p ci k", p=P),
                        in_=outT[:].rearrange("p (ci k) -> p ci k", k=K),
                    )
                    if si + 1 < s:
                        planes_cur = planes_next
    nc.compile()
    return nc


def _get_compiled():
    global _compiled
    if _compiled is None:
        _compiled = _build_nc()
    return _compiled


def kernel(feats, gt_seg_map):
    from concourse.bass_utils import run_bass_kernel_spmd

    feats = np.asarray(feats, dtype=np.float32).reshape(B, C, HW)
    gt = np.asarray(gt_seg_map).astype(np.int32).reshape(B, HW)

    nc = _get_compiled()
    in_maps = []
    for i in range(N_CORES):
        in_maps.append(
            {
                "feats": feats[i * S : (i + 1) * S],
                "gt_seg_map": gt[i * S : (i + 1) * S],
            }
        )
    res = run_bass_kernel_spmd(nc, in_maps, core_ids=list(range(N_CORES)))
    parts = [res.results[i]["out"] for i in range(N_CORES)]  # each [S, C, K]
    full = np.concatenate(parts, axis=0)  # [B, C, K]
    return full[..., None].astype(np.float32)  # [B, C, K, 1]
